# revision 22
# baseline (speedup 1.0000x reference)
"""Trainium2 Bass kernel for nn_BiLSTMDecoderModel.

Strategy (8 NeuronCores, data-parallel over batch, B=128 -> 16 rows/core):
  * backward LSTM: only b_hs[0] is consumed downstream == ONE cell step on x_0.
  * forward LSTM: 256-step scan, h-stationary recurrent matmul with 4-way PE
    column tiling; xp injected into PSUM first (identity-stationary matmul).
  * gate nonlinearities collapse to tanh via sigmoid(x)=(tanh(x/2)+1)/2 with
    scale factors folded into host-preprocessed weights (carried z=2c, h'=2h).
  * phase A (gather + tanh + transpose + input projection) is INTERLEAVED with
    the scan (per-nb parts spread across each 8-step window) and feeds the scan
    directly from SBUF; dummy matmuls fill remaining PE idle so the HAM clock
    gate stays at K=8/8 (2.4 GHz) instead of 1.2 GHz.
  * decoder: gi GEMM computed once; per-class rows obtained via SBUF->SBUF DMA
    row moves instead of recomputing the GEMM per class.
"""

import sys

sys.path.insert(0, "/opt/trn_rl_repo")

import numpy as np
import ml_dtypes

import concourse.bass as bass
import concourse.mybir as mybir
import concourse.tile as tile
from concourse import bacc
from concourse.bass_utils import run_bass_kernel_spmd
from concourse.masks import make_identity

V, E, H, NCLS = 100000, 300, 512, 6
B, S = 128, 256
NC = 8
BL = B // NC  # 16
G4 = 4 * H  # 2048
G3 = 3 * 2 * H  # 3072
H2 = 2 * H  # 1024

DN = 2    # full-size warm-keeper matmuls after the per-step MM block
DN2 = 3   # full-size warm-keepers after the transposes (cover the vec chain)

f32 = mybir.dt.float32
bf16 = mybir.dt.bfloat16
i32 = mybir.dt.int32
Tanh = mybir.ActivationFunctionType.Tanh
Exp = mybir.ActivationFunctionType.Exp
Ln = mybir.ActivationFunctionType.Ln
Ident = mybir.ActivationFunctionType.Identity
ADD = mybir.AluOpType.add
SUB = mybir.AluOpType.subtract
MUL = mybir.AluOpType.mult
MAX = mybir.AluOpType.max

_cache = {}


def _bf(x):
    return np.ascontiguousarray(x.astype(ml_dtypes.bfloat16))


def _build_program():
    nc = bacc.Bacc(
        "TRN2", target_bir_lowering=False, debug=False, enable_asserts=False,
        num_devices=NC,
    )
    embedW_d = nc.dram_tensor("embedW", [V, E], f32, kind="ExternalInput").ap()
    idx_d = nc.dram_tensor("idx", [128, 32], i32, kind="ExternalInput").ap()
    wihT_d = nc.dram_tensor("wihT", [304, G4], bf16, kind="ExternalInput").ap()
    bwihT_d = nc.dram_tensor("bwihT", [304, G4], bf16, kind="ExternalInput").ap()
    whhT_d = nc.dram_tensor("whhT", [H, G4], bf16, kind="ExternalInput").ap()
    dwhhT_d = nc.dram_tensor("dwhhT", [1028, G3], bf16, kind="ExternalInput").ap()
    dwihT_d = nc.dram_tensor("dwihT", [516, G3], bf16, kind="ExternalInput").ap()
    ecw_d = nc.dram_tensor("ecw", [NCLS, H], f32, kind="ExternalInput").ap()
    clsT_d = nc.dram_tensor("clsT", [1028, 2], bf16, kind="ExternalInput").ap()
    out_d = nc.dram_tensor("out", [NCLS, BL, 2], f32, kind="ExternalOutput").ap()

    with tile.TileContext(nc) as tc:
        _emit(nc, tc, embedW_d, idx_d, wihT_d, bwihT_d, whhT_d, dwhhT_d,
              dwihT_d, ecw_d, clsT_d, out_d)
    nc.compile()
    return nc


def _emit(nc, tc, embedW_d, idx_d, wihT_d, bwihT_d, whhT_d, dwhhT_d, dwihT_d,
          ecw_d, clsT_d, out_d):
    def pool(**kw):
        return tc.alloc_tile_pool(**kw)

    const = pool(name="const", bufs=1)
    dramp = pool(name="dram", bufs=1, space="DRAM")

    # ---- persistent SBUF constants ----
    ident = const.tile([128, 128], f32, tag="ident", name="ident")
    make_identity(nc, ident[:])
    identb = const.tile([128, 128], bf16, tag="identb", name="identb")
    make_identity(nc, identb[:])
    i16 = const.tile([16, 16], bf16, tag="i16", name="i16")
    make_identity(nc, i16[:])
    ones1 = const.tile([1, 16], bf16, tag="ones1", name="ones1")
    nc.gpsimd.memset(ones1[:], 1.0)
    onesr = const.tile([1, 128], bf16, tag="onesr", name="onesr")
    nc.gpsimd.memset(onesr[:], 1.0)
    biasf = const.tile([1, 2048], bf16, tag="biasf", name="biasf")
    biasb = const.tile([1, 2048], bf16, tag="biasb", name="biasb")
    bias_stat = const.tile([4, 16], bf16, tag="bias_stat", name="bias_stat")
    nc.gpsimd.memset(bias_stat[:], 0.0)
    nc.gpsimd.memset(bias_stat[0:1, :], 1.0)
    junkW = const.tile([128, 512], bf16, tag="junkW", name="junkW")
    nc.gpsimd.memset(junkW[:], 0.0)

    def tr(out_ap, in_ap, pin):
        nc.tensor.transpose(out_ap, in_ap, ident[0:pin, 0:pin])

    idx_sb = const.tile([128, 32], i32, tag="idx", name="idx")
    nc.sync.dma_start(idx_sb[:], idx_d[:])

    wih_sb = [const.tile([128, G4], bf16, tag=f"wih{k}", name=f"wih{k}") for k in range(3)]
    bwih_sb = [const.tile([128, G4], bf16, tag=f"bwih{k}", name=f"bwih{k}") for k in range(3)]
    for k in range(2):
        nc.sync.dma_start(wih_sb[k][:], wihT_d[128 * k:128 * (k + 1), :])
        nc.sync.dma_start(bwih_sb[k][:], bwihT_d[128 * k:128 * (k + 1), :])
    nc.sync.dma_start(wih_sb[2][0:44, :], wihT_d[256:300, :])
    nc.sync.dma_start(bwih_sb[2][0:44, :], bwihT_d[256:300, :])
    nc.sync.dma_start(biasf[:], wihT_d[300:301, :])
    nc.sync.dma_start(biasb[:], bwihT_d[300:301, :])

    whh_sb = [const.tile([128, G4], bf16, tag=f"whh{k}", name=f"whh{k}") for k in range(4)]
    for k in range(4):
        nc.sync.dma_start(whh_sb[k][:], whhT_d[128 * k:128 * (k + 1), :])

    dwhh_sb = [const.tile([128, G3], bf16, tag=f"dwhh{k}", name=f"dwhh{k}") for k in range(9)]
    for k in range(8):
        nc.sync.dma_start(dwhh_sb[k][:], dwhhT_d[128 * k:128 * (k + 1), :])
    nc.sync.dma_start(dwhh_sb[8][0:4, :], dwhhT_d[1024:1028, :])

    dwih_sb = [const.tile([128, G3], bf16, tag=f"dwih{k}", name=f"dwih{k}") for k in range(5)]
    for k in range(4):
        nc.sync.dma_start(dwih_sb[k][:], dwihT_d[128 * k:128 * (k + 1), :])
    nc.sync.dma_start(dwih_sb[4][0:4, :], dwihT_d[512:516, :])

    cls_sb = [const.tile([128, 2], bf16, tag=f"cls{k}", name=f"cls{k}") for k in range(9)]
    for k in range(8):
        nc.sync.dma_start(cls_sb[k][:], clsT_d[128 * k:128 * (k + 1), :])
    nc.sync.dma_start(cls_sb[8][0:4, :], clsT_d[1024:1028, :])

    bH = const.tile([128, 64], bf16, tag="bH", name="bH")  # backward-cell h' (2h)

    # ---- pools (released LIFO; pH outlives the scan into the decoder) ----
    pH = pool(name="pH", bufs=2)
    pA = pool(name="pA", bufs=3)       # gather / tanh / embT tiles
    pXP = pool(name="pXP", bufs=3)     # xp [128, G4] bf16 per 8-step window
    pST = pool(name="pST", bufs=6)     # per-step [16, G4] stage (partition 0)
    pT = pool(name="pT", bufs=2)
    pZ = pool(name="pZ", bufs=2)
    pW = pool(name="pW", bufs=2)
    pAps2 = pool(name="pAps2", bufs=1, space="PSUM")   # A transposes (1 bank)
    pApsx = pool(name="pApsx", bufs=2, space="PSUM")   # per-nb xproj psum (2 banks)
    pPS = pool(name="pPS", bufs=2, space="PSUM")       # psg / pstT
    pJ = pool(name="pJ", bufs=1, space="PSUM")         # junk bank for warmers

    emb_sb = {}

    def emit_gather(m):
        g_t = pA.tile([128, 304], f32, tag="gath", name="gath")
        nc.gpsimd.indirect_dma_start(
            out=g_t[:, 0:E],
            out_offset=None,
            in_=embedW_d[:],
            in_offset=bass.IndirectOffsetOnAxis(ap=idx_sb[:, m:m + 1], axis=0),
        )
        return g_t

    gath = {}

    A_th = {}

    def emit_A_tanh(m):
        # deferred to the post-vec slot: off the step-critical scalar queue
        th = pA.tile([128, 304], f32, tag="th", name="th")
        nc.scalar.activation(th[:, 0:E], gath[m][:, 0:E], Tanh)
        A_th[m] = th

    def emit_A_tr(m):
        th = A_th.pop(m)
        pst = pAps2.tile([128, 384], f32, tag="pst", name="pst")
        tr(pst[0:128, 0:128], th[:, 0:128], 128)
        tr(pst[0:128, 128:256], th[:, 128:256], 128)
        tr(pst[0:44, 256:384], th[:, 256:300], 128)
        embT_a = pA.tile([128, 256], bf16, tag="embTa", name="embTa")
        nc.vector.tensor_copy(embT_a[:], pst[:, 0:256])
        embT_b = pA.tile([48, 128], bf16, tag="embTb", name="embTb")
        nc.vector.tensor_copy(embT_b[0:44, :], pst[0:44, 256:384])
        emb_sb[m] = (embT_a, embT_b)

    def emit_A_nb_mm(m, nb):
        embT_a, embT_b = emb_sb[m]
        nsl = slice(512 * nb, 512 * (nb + 1))
        psx = pApsx.tile([128, 512], f32, tag="psx", name="psx")
        nc.tensor.matmul(psx[:], embT_a[:, 0:128], wih_sb[0][:, nsl],
                         start=True, stop=False)
        nc.tensor.matmul(psx[:], embT_a[:, 128:256], wih_sb[1][:, nsl],
                         start=False, stop=False)
        nc.tensor.matmul(psx[:], embT_b[0:44, :], wih_sb[2][0:44, nsl],
                         start=False, stop=False)
        nc.tensor.matmul(psx[:], onesr[:, 0:128], biasf[0:1, nsl],
                         start=False, stop=True)
        return psx

    def emit_A_nb_copy(m, nb, psx):
        nsl = slice(512 * nb, 512 * (nb + 1))
        if nb % 2 == 0:
            nc.vector.tensor_copy(xp_tiles[m][:, nsl], psx[:])
        else:
            nc.scalar.activation(xp_tiles[m][:, nsl], psx[:], Ident)

    def emit_junk(n, jpool=None):
        # dense warm-keeper burst: one accumulation group so the MMs stream
        # back-to-back (start=True per-MM would serialize on bank drain)
        jp = (jpool or pJ).tile([128, 512], f32, tag="junk", name="junk")
        for k in range(n):
            nc.tensor.matmul(jp[:], identb[:], junkW[:],
                             start=(k == 0), stop=(k == n - 1))

    # ======== lead-in: A(0), A(1) full + backward cell ========
    xp_tiles = {}
    for m in (0, 1, 2):
        gath[m] = emit_gather(m)
    for m in (0, 1):
        xp_tiles[m] = pXP.tile([128, G4], bf16, tag="xp", name="xp")
        emit_A_tanh(m)
        emit_A_tr(m)
        for nb in range(4):
            psx = emit_A_nb_mm(m, nb)
            emit_A_nb_copy(m, nb, psx)

    # backward LSTM single cell on x_0 (h=c=0), reusing pPS banks
    emb0_a, emb0_b = emb_sb[0]
    bps = pPS.tile([128, 512], f32, tag="psg", name="bps")
    for j in range(4):
        ns = slice(512 * j, 512 * (j + 1))
        o = bps[32 * j:32 * j + 16, :]
        tp = (0, 32 * j)
        nc.tensor.matmul(o, emb0_a[:, 0:16], bwih_sb[0][:, ns],
                         start=True, stop=False, tile_position=tp)
        nc.tensor.matmul(o, emb0_a[:, 128:144], bwih_sb[1][:, ns],
                         start=False, stop=False, tile_position=tp)
        nc.tensor.matmul(o, emb0_b[0:44, 0:16], bwih_sb[2][0:44, ns],
                         start=False, stop=False, tile_position=tp)
        nc.tensor.matmul(o, onesr[:, 0:16], biasb[0:1, ns],
                         start=False, stop=True, tile_position=tp)
    bT = pA.tile([128, 512], f32, tag="bT", name="bT")
    nc.scalar.activation(bT[0:112, :], bps[0:112, :], Tanh)
    bpt = pPS.tile([128, 448], f32, tag="pstT", name="bpt")
    for k in range(4):
        tr(bpt[:, 112 * k:112 * (k + 1)], bT[0:112, 128 * k:128 * (k + 1)], 112)
    bv = bpt[:].rearrange("p (c w) -> p c w", w=112)
    btip = pA.tile([128, 64], f32, tag="btip", name="btip")
    nc.vector.tensor_scalar_add(
        out=btip[:].rearrange("p (c w) -> p c w", w=16),
        in0=bv[:, :, 0:16], scalar1=1.0)
    bzv = pA.tile([128, 64], f32, tag="bzv", name="bzv")
    nc.vector.tensor_tensor(
        out=bzv[:].rearrange("p (c w) -> p c w", w=16),
        in0=btip[:].rearrange("p (c w) -> p c w", w=16),
        in1=bv[:, :, 64:80], op=MUL)
    btc = pA.tile([128, 64], f32, tag="btc", name="btc")
    nc.scalar.activation(btc[:], bzv[:], Tanh, scale=0.5)
    nc.vector.scalar_tensor_tensor(
        out=bH[:].rearrange("p (c w) -> p c w", w=16),
        in0=bv[:, :, 96:112], scalar=1.0,
        in1=btc[:].rearrange("p (c w) -> p c w", w=16),
        op0=ADD, op1=MUL)

    # ======== forward scan, 256 steps, phase A interleaved ========
    z_prev = [pZ.tile([128, 32], f32, tag=f"z{h}", name=f"z{h}") for h in range(2)]
    H_prev = [pH.tile([128, 32], bf16, tag=f"H{h}", name=f"H{h}") for h in range(2)]
    for h in range(2):
        nc.vector.memset(z_prev[h][:], 0.0)
        nc.vector.memset(H_prev[h][:], 0.0)
    emit_junk(10)

    def emit_stage(t):
        st = pST.tile([16, G4], bf16, tag="st", name="st")
        sub = t % 8
        nc.sync.dma_start(st[:], xp_tiles[t // 8][16 * sub:16 * sub + 16, :])
        return st

    stage = {t: emit_stage(t) for t in range(4)}

    def emit_inject(t, psg):
        xp_src = stage.pop(t)
        for j in range(4):
            nc.tensor.matmul(
                psg[32 * j:32 * j + 16, :], i16[:],
                xp_src[:, 512 * j:512 * (j + 1)],
                start=True, stop=False, tile_position=(0, 32 * j))

    psg_cur = pPS.tile([128, 512], f32, tag="psg", name="psg")
    emit_inject(0, psg_cur)

    for t in range(S):
        m = t // 8
        sub = t % 8
        has_A = m + 2 <= 31
        a_mm = has_A and 3 <= sub <= 6
        if t + 4 < S:
            stage[t + 4] = emit_stage(t + 4)

        # ---- gate matmuls for step t (xp inject already accumulated) ----
        for kc in range(4):
            hh = H_prev[kc // 2]
            col = 16 * (kc % 2)
            for j in range(4):
                nc.tensor.matmul(
                    psg_cur[32 * j:32 * j + 16, :], hh[:, col:col + 16],
                    whh_sb[kc][:, 512 * j:512 * (j + 1)],
                    start=False, stop=(kc == 3), tile_position=(0, 32 * j))
        emit_junk(DN)

        # ---- nonlinearity + transposes, split in hidden halves ----
        T_t = pT.tile([128, 512], f32, tag="T", name="T")
        nc.scalar.activation(T_t[0:112, 0:256], psg_cur[0:112, 0:256], Tanh)
        nc.scalar.activation(T_t[0:112, 256:512], psg_cur[0:112, 256:512], Tanh)
        pstT = pPS.tile([128, 448], f32, tag="pstT", name="pstT")
        for k in range(4):
            tr(pstT[:, 112 * k:112 * (k + 1)], T_t[0:112, 128 * k:128 * (k + 1)], 112)

        # ---- pre-issue next step's inject; fill PE with A matmuls / junk ----
        psg_nxt = None
        if t + 1 < S:
            psg_nxt = pPS.tile([128, 512], f32, tag="psg", name="psg")
            emit_inject(t + 1, psg_nxt)
        a_psx = None
        if has_A and a_mm:
            a_psx = emit_A_nb_mm(m + 2, sub - 3)
        if has_A and sub == 2:
            emit_A_tr(m + 2)  # PE transposes; A-tanh ran in sub1's tail
        if not a_mm:
            emit_junk(DN2)

        # ---- vec chains per hidden half ----
        Tv = pstT[:].rearrange("p (c w) -> p c w", w=112)
        z_new = [pZ.tile([128, 32], f32, tag=f"z{h}", name=f"z{h}") for h in range(2)]
        H_new = [pH.tile([128, 32], bf16, tag=f"H{h}", name=f"H{h}") for h in range(2)]
        tc_ts = []
        for h in range(2):
            cs = slice(2 * h, 2 * h + 2)
            ti, tf = Tv[:, cs, 0:16], Tv[:, cs, 32:48]
            tg = Tv[:, cs, 64:80]
            zpv = z_prev[h][:].rearrange("p (c w) -> p c w", w=16)
            a_t = pW.tile([128, 32], f32, tag=f"a{h}", name=f"a{h}")
            av = a_t[:].rearrange("p (c w) -> p c w", w=16)
            nc.vector.scalar_tensor_tensor(out=av, in0=tf, scalar=1.0, in1=zpv,
                                           op0=ADD, op1=MUL)
            tip = pW.tile([128, 32], f32, tag=f"tip{h}", name=f"tip{h}")
            tipv = tip[:].rearrange("p (c w) -> p c w", w=16)
            nc.vector.tensor_scalar_add(out=tipv, in0=ti, scalar1=1.0)
            v_t = pW.tile([128, 32], f32, tag=f"v{h}", name=f"v{h}")
            vv = v_t[:].rearrange("p (c w) -> p c w", w=16)
            nc.vector.tensor_tensor(out=vv, in0=tipv, in1=tg, op=MUL)
            nc.vector.scalar_tensor_tensor(out=z_new[h][:], in0=a_t[:], scalar=0.5,
                                           in1=v_t[:], op0=MUL, op1=ADD)
            tc_t = pW.tile([128, 32], f32, tag=f"tc{h}", name=f"tc{h}")
            nc.scalar.activation(tc_t[:], z_new[h][:], Tanh, scale=0.5)
            tc_ts.append(tc_t)
        for h in range(2):
            cs = slice(2 * h, 2 * h + 2)
            to = Tv[:, cs, 96:112]
            nc.vector.scalar_tensor_tensor(
                out=H_new[h][:].rearrange("p (c w) -> p c w", w=16),
                in0=to, scalar=1.0,
                in1=tc_ts[h][:].rearrange("p (c w) -> p c w", w=16),
                op0=ADD, op1=MUL)

        # ---- deferred off-chain A work (runs while next step computes) ----
        if has_A:
            if sub == 0 and m + 3 <= 31:
                gath[m + 3] = emit_gather(m + 3)
            elif sub == 1:
                xp_tiles[m + 2] = pXP.tile([128, G4], bf16, tag="xp", name="xp")
                emit_A_tanh(m + 2)
            elif a_mm:
                emit_A_nb_copy(m + 2, sub - 3, a_psx)

        z_prev, H_prev = z_new, H_new
        if psg_nxt is not None:
            psg_cur = psg_nxt

    pJ.release()
    pPS.release()
    pApsx.release()
    pAps2.release()
    pW.release()
    pZ.release()
    pT.release()
    pST.release()
    pXP.release()
    pA.release()

    # ======== decoder (6 GRU steps + logits + log_softmax) ========
    pD = pool(name="pD", bufs=1)
    pDgi = pool(name="pDgi", bufs=1, space="PSUM")

    ce_t = pD.tile([NCLS, H], f32, tag="ce", name="ce")
    nc.sync.dma_start(ce_t[:], ecw_d[:])
    ce2 = pD.tile([NCLS, H], f32, tag="ce2", name="ce2")
    nc.scalar.activation(ce2[:], ce_t[:], Tanh)
    psc = pDgi.tile([128, 24], f32, tag="psc", name="psc")
    for k in range(4):
        tr(psc[:, 6 * k:6 * (k + 1)], ce2[0:NCLS, 128 * k:128 * (k + 1)], NCLS)
    ceT = pD.tile([128, 24], bf16, tag="ceT", name="ceT")
    nc.vector.tensor_copy(ceT[:], psc[:])

    psgi = pDgi.tile([NCLS, G3], f32, tag="psgi", name="psgi")
    for ng in range(6):
        ns = slice(512 * ng, 512 * (ng + 1))
        for kc in range(4):
            nc.tensor.matmul(psgi[:, ns], ceT[:, 6 * kc:6 * (kc + 1)],
                             dwih_sb[kc][:, ns], start=(kc == 0), stop=False)
        nc.tensor.matmul(psgi[:, ns], bias_stat[0:4, 0:NCLS],
                         dwih_sb[4][0:4, ns], start=False, stop=True)
    gi_sb = pD.tile([NCLS, G3], bf16, tag="gi", name="gi")
    nc.scalar.activation(gi_sb[:], psgi[:], Ident)

    # transposed gi_n (per-partition bias for the n-gate)
    psgT = pDgi.tile([128, 48], bf16, tag="psgT", name="psgT")
    for gc in range(8):
        nc.tensor.transpose(psgT[:, 6 * gc:6 * (gc + 1)],
                            gi_sb[0:NCLS, 2048 + 128 * gc:2048 + 128 * (gc + 1)],
                            identb[0:NCLS, 0:NCLS])
    giT = pD.tile([128, 48], f32, tag="giT", name="giT")
    nc.vector.tensor_copy(giT[:], psgT[:])

    # per-class gi rows at partition 0, via DRAM bounce (avoids recompute)
    gi_dram = dramp.tile([NCLS, G3], bf16, tag="gid", name="gid")
    nc.sync.dma_start(gi_dram[:], gi_sb[:])
    gi_row = pD.tile([1, NCLS * G3], bf16, tag="girow", name="girow")
    for c in range(NCLS):
        nc.sync.dma_start(gi_row[0:1, G3 * c:G3 * (c + 1)],
                          gi_dram[c:c + 1, :])
    pDgi.release()
    pDps = pool(name="pDps", bufs=1, space="PSUM")
    pJ2 = pool(name="pJ2", bufs=1, space="PSUM")

    Hd = pD.tile([128, 128], bf16, tag="Hd", name="Hd")
    nc.vector.tensor_scalar_mul(Hd[:, 0:32], H_prev[0][:], 0.5)
    nc.vector.tensor_scalar_mul(Hd[:, 32:64], H_prev[1][:], 0.5)
    nc.vector.tensor_scalar_mul(Hd[:, 64:128], bH[:], 0.5)

    l_all = pD.tile([16, 12], f32, tag="lall", name="lall")

    for c in range(NCLS):
        psd0 = pDps.tile([128, 512], f32, tag="psd0", name="psd0")
        psd1 = pDps.tile([128, 512], f32, tag="psd1", name="psd1")
        for kc in range(8):
            lh = Hd[:, 16 * kc:16 * (kc + 1)]
            for ng in range(6):
                ps, j = (psd0, ng) if ng < 4 else (psd1, ng - 4)
                nc.tensor.matmul(
                    ps[32 * j:32 * j + 16, :], lh,
                    dwhh_sb[kc][:, 512 * ng:512 * (ng + 1)],
                    start=(kc == 0), stop=False, tile_position=(0, 32 * j))
        for ng in range(6):
            ps, j = (psd0, ng) if ng < 4 else (psd1, ng - 4)
            nc.tensor.matmul(
                ps[32 * j:32 * j + 16, :], bias_stat[0:4, :],
                dwhh_sb[8][0:4, 512 * ng:512 * (ng + 1)],
                start=False, stop=(ng >= 4), tile_position=(0, 32 * j))
        for ng in range(4):
            nc.tensor.matmul(
                psd0[32 * ng:32 * ng + 16, :], ones1[:],
                gi_row[0:1, G3 * c + 512 * ng:G3 * c + 512 * (ng + 1)],
                start=False, stop=True, tile_position=(0, 32 * ng))

        Trz = pD.tile([128, 512], f32, tag="Trz", name="Trz")
        nc.scalar.activation(Trz[0:112, :], psd0[0:112, :], Tanh)
        pstz = pDps.tile([128, 448], f32, tag="pstz", name="pstz")
        for k in range(4):
            tr(pstz[:, 112 * k:112 * (k + 1)], Trz[0:112, 128 * k:128 * (k + 1)], 112)
        hn_sb = pD.tile([48, 512], f32, tag="hn", name="hn")
        nc.vector.tensor_copy(hn_sb[:], psd1[0:48, :])
        psn = pDps.tile([128, 192], f32, tag="psn", name="psn")
        for k in range(4):
            tr(psn[:, 48 * k:48 * (k + 1)], hn_sb[0:48, 128 * k:128 * (k + 1)], 48)
        emit_junk(10, pJ2)

        zv = pstz[:].rearrange("p (c w) -> p c w", w=112)
        nv = psn[:].rearrange("p (c w) -> p c w", w=48)
        trp = pD.tile([128, 128], f32, tag="trp", name="trp")
        trpv = trp[:].rearrange("p (g w) -> p g w", w=16)
        sn_t = pD.tile([128, 128], f32, tag="sn", name="sn")
        snv = sn_t[:].rearrange("p (g w) -> p g w", w=16)
        nT = pD.tile([128, 128], f32, tag="nT", name="nT")
        for s in range(2):
            nc.vector.tensor_scalar_add(
                out=trpv[:, 4 * s:4 * s + 4, :],
                in0=zv[:, :, 32 * s:32 * s + 16], scalar1=1.0)
            nc.vector.tensor_tensor(
                out=snv[:, 4 * s:4 * s + 4, :],
                in0=trpv[:, 4 * s:4 * s + 4, :],
                in1=nv[:, :, 32 * s:32 * s + 16], op=MUL)
        nTv = nT[:].rearrange("p (g w) -> p g w", w=16)
        for g in range(8):
            nc.scalar.activation(nTv[:, g, :], snv[:, g, :], Tanh, scale=0.5,
                                 bias=giT[:, 6 * g + c:6 * g + c + 1])
        d_t = pD.tile([128, 128], f32, tag="dt", name="dt")
        nc.vector.tensor_tensor(out=d_t[:], in0=Hd[:], in1=nT[:], op=SUB)
        e_t = pD.tile([128, 128], f32, tag="et", name="et")
        ev = e_t[:].rearrange("p (g w) -> p g w", w=16)
        dv = d_t[:].rearrange("p (g w) -> p g w", w=16)
        for s in range(2):
            nc.vector.scalar_tensor_tensor(
                out=ev[:, 4 * s:4 * s + 4, :],
                in0=zv[:, :, 64 + 32 * s:80 + 32 * s], scalar=1.0,
                in1=dv[:, 4 * s:4 * s + 4, :], op0=ADD, op1=MUL)
        hn2 = pD.tile([128, 128], f32, tag="hn2", name="hn2")
        nc.vector.scalar_tensor_tensor(out=hn2[:], in0=e_t[:], scalar=0.5,
                                       in1=nT[:], op0=MUL, op1=ADD)
        Hd_new = pD.tile([128, 128], bf16, tag="Hd", name="Hd")
        nc.scalar.activation(Hd_new[:], hn2[:], Tanh)

        psl = pDps.tile([16, 2], f32, tag="psl", name="psl")
        for kc in range(8):
            nc.tensor.matmul(psl[:], Hd_new[:, 16 * kc:16 * (kc + 1)],
                             cls_sb[kc][:, 0:2], start=(kc == 0), stop=False)
        nc.tensor.matmul(psl[:], bias_stat[0:4, :], cls_sb[8][0:4, 0:2],
                         start=False, stop=True)
        nc.vector.tensor_copy(l_all[:, 2 * c:2 * c + 2], psl[:])
        Hd = Hd_new

    la = l_all[:].rearrange("p (c t) -> p c t", t=2)
    mx = pD.tile([16, 6], f32, tag="mx", name="mx")
    nc.vector.tensor_tensor(out=mx[:].rearrange("p (c o) -> p c o", o=1),
                            in0=la[:, :, 0:1], in1=la[:, :, 1:2], op=MAX)
    d0 = pD.tile([16, 12], f32, tag="d0", name="d0")
    d0v = d0[:].rearrange("p (c t) -> p c t", t=2)
    mxb = mx[:].rearrange("p (c o) -> p c o", o=1).to_broadcast([16, 6, 2])
    nc.vector.tensor_tensor(out=d0v, in0=la, in1=mxb, op=SUB)
    ex = pD.tile([16, 12], f32, tag="ex", name="ex")
    nc.scalar.activation(ex[:], d0[:], Exp)
    se = pD.tile([16, 6], f32, tag="se", name="se")
    nc.vector.tensor_reduce(out=se[:].rearrange("p (c o) -> p c o", o=1),
                            in_=ex[:].rearrange("p (c t) -> p c t", t=2),
                            op=ADD, axis=mybir.AxisListType.X)
    ls = pD.tile([16, 6], f32, tag="ls", name="ls")
    nc.scalar.activation(ls[:], se[:], Ln)
    ov = pD.tile([16, 12], f32, tag="ov", name="ov")
    lsb = ls[:].rearrange("p (c o) -> p c o", o=1).to_broadcast([16, 6, 2])
    nc.vector.tensor_tensor(out=ov[:].rearrange("p (c t) -> p c t", t=2),
                            in0=d0v, in1=lsb, op=SUB)
    nc.sync.dma_start(out_d[:].rearrange("c b t -> b c t"),
                      ov[:].rearrange("p (c t) -> p c t", t=2))

    pJ2.release()
    pDps.release()
    pD.release()
    pH.release()
    dramp.release()
    const.release()


def _prep_inputs(seq, classes, embed_W, embed_class_W, f_Wih, f_Whh, f_b,
                 b_Wih, b_Whh, b_b, d_Wih, d_Whh, d_bih, d_bhh, cls_W, cls_b):
    seq = np.asarray(seq)
    s4 = np.concatenate([np.full(H, 0.5), np.full(H, 0.5), np.ones(H),
                         np.full(H, 0.5)]).astype(np.float32)
    s3 = np.concatenate([np.full(H2, 0.5), np.full(H2, 0.5),
                         np.ones(H2)]).astype(np.float32)

    def padrows(a, rows):
        out = np.zeros((rows, a.shape[1]), np.float32)
        out[:a.shape[0]] = a
        return out

    wihT = padrows(np.concatenate(
        [(f_Wih * s4[:, None]).T, (f_b * s4)[None, :]], axis=0), 304)
    bwihT = padrows(np.concatenate(
        [(b_Wih * s4[:, None]).T, (b_b * s4)[None, :]], axis=0), 304)
    whhT = ((f_Whh * s4[:, None]) * 0.5).T.astype(np.float32)
    dwhhT = padrows(np.concatenate(
        [(d_Whh * s3[:, None]).T, (d_bhh * s3)[None, :]], axis=0), 1028)
    dwihT = padrows(np.concatenate(
        [(d_Wih * s3[:, None]).T, (d_bih * s3)[None, :]], axis=0), 516)
    clsT = padrows(np.concatenate(
        [np.asarray(cls_W, np.float32).T, np.asarray(cls_b, np.float32)[None, :]],
        axis=0), 1028)
    ecw = np.asarray(embed_class_W, np.float32)[np.asarray(classes)]

    shared = {
        "embedW": np.ascontiguousarray(np.asarray(embed_W, np.float32)),
        "wihT": _bf(wihT), "bwihT": _bf(bwihT), "whhT": _bf(whhT),
        "dwhhT": _bf(dwhhT), "dwihT": _bf(dwihT),
        "ecw": np.ascontiguousarray(ecw),
        "clsT": _bf(clsT),
    }
    in_maps = []
    for c in range(NC):
        tok = np.asarray(seq[BL * c:BL * (c + 1), :], np.int32)  # [16, 256]
        idx = np.ascontiguousarray(
            tok.T.reshape(S * BL).reshape(32, 128).T.astype(np.int32))
        m = dict(shared)
        m["idx"] = idx
        in_maps.append(m)
    return in_maps


def kernel(**inputs):
    if "nc" not in _cache:
        _cache["nc"] = _build_program()
    nc = _cache["nc"]
    in_maps = _prep_inputs(**inputs)
    import os
    trace = bool(int(os.environ.get("BK_TRACE", "0")))
    res = run_bass_kernel_spmd(nc, in_maps, core_ids=list(range(NC)),
                               trace=trace)
    _cache["last_result"] = res
    outs = [res.results[c]["out"] for c in range(NC)]
    return np.concatenate(outs, axis=1).astype(np.float32)


# revision 23
# speedup vs baseline: 1.0709x; 1.0709x over previous
"""Trainium2 Bass kernel for nn_BiLSTMDecoderModel.

Strategy (8 NeuronCores, data-parallel over batch, B=128 -> 16 rows/core):
  * backward LSTM: only b_hs[0] is consumed downstream == ONE cell step on x_0.
  * forward LSTM: 256-step scan, h-stationary recurrent matmul with 4-way PE
    column tiling; xp injected into PSUM first (identity-stationary matmul).
  * gate nonlinearities collapse to tanh via sigmoid(x)=(tanh(x/2)+1)/2 with
    scale factors folded into host-preprocessed weights (carried z=2c, h'=2h).
  * phase A (gather + tanh + transpose + input projection) is INTERLEAVED with
    the scan (per-nb parts spread across each 8-step window) and feeds the scan
    directly from SBUF; dummy matmuls fill remaining PE idle so the HAM clock
    gate stays at K=8/8 (2.4 GHz) instead of 1.2 GHz.
  * decoder: gi GEMM computed once; per-class rows obtained via SBUF->SBUF DMA
    row moves instead of recomputing the GEMM per class.
"""

import sys

sys.path.insert(0, "/opt/trn_rl_repo")

import numpy as np
import ml_dtypes

import concourse.bass as bass
import concourse.mybir as mybir
import concourse.tile as tile
from concourse import bacc
from concourse.bass_utils import run_bass_kernel_spmd
from concourse.masks import make_identity

V, E, H, NCLS = 100000, 300, 512, 6
B, S = 128, 256
NC = 8
BL = B // NC  # 16
G4 = 4 * H  # 2048
G3 = 3 * 2 * H  # 3072
H2 = 2 * H  # 1024

DN = 3    # full-size warm-keeper matmuls after the per-step MM block
DN2 = 5   # full-size warm-keepers after the transposes (cover the vec chain)

f32 = mybir.dt.float32
bf16 = mybir.dt.bfloat16
i32 = mybir.dt.int32
Tanh = mybir.ActivationFunctionType.Tanh
Exp = mybir.ActivationFunctionType.Exp
Ln = mybir.ActivationFunctionType.Ln
Ident = mybir.ActivationFunctionType.Identity
ADD = mybir.AluOpType.add
SUB = mybir.AluOpType.subtract
MUL = mybir.AluOpType.mult
MAX = mybir.AluOpType.max

_cache = {}


def _bf(x):
    return np.ascontiguousarray(x.astype(ml_dtypes.bfloat16))


def _build_program():
    nc = bacc.Bacc(
        "TRN2", target_bir_lowering=False, debug=False, enable_asserts=False,
        num_devices=NC,
    )
    embedW_d = nc.dram_tensor("embedW", [V, E], f32, kind="ExternalInput").ap()
    idx_d = nc.dram_tensor("idx", [128, 32], i32, kind="ExternalInput").ap()
    wihT_d = nc.dram_tensor("wihT", [304, G4], bf16, kind="ExternalInput").ap()
    bwihT_d = nc.dram_tensor("bwihT", [304, G4], bf16, kind="ExternalInput").ap()
    whhT_d = nc.dram_tensor("whhT", [H, G4], bf16, kind="ExternalInput").ap()
    dwhhT_d = nc.dram_tensor("dwhhT", [1028, G3], bf16, kind="ExternalInput").ap()
    dwihT_d = nc.dram_tensor("dwihT", [516, G3], bf16, kind="ExternalInput").ap()
    ecw_d = nc.dram_tensor("ecw", [NCLS, H], f32, kind="ExternalInput").ap()
    clsT_d = nc.dram_tensor("clsT", [1028, 2], bf16, kind="ExternalInput").ap()
    out_d = nc.dram_tensor("out", [NCLS, BL, 2], f32, kind="ExternalOutput").ap()

    with tile.TileContext(nc) as tc:
        _emit(nc, tc, embedW_d, idx_d, wihT_d, bwihT_d, whhT_d, dwhhT_d,
              dwihT_d, ecw_d, clsT_d, out_d)
    nc.compile()
    return nc


def _emit(nc, tc, embedW_d, idx_d, wihT_d, bwihT_d, whhT_d, dwhhT_d, dwihT_d,
          ecw_d, clsT_d, out_d):
    def pool(**kw):
        return tc.alloc_tile_pool(**kw)

    const = pool(name="const", bufs=1)
    dramp = pool(name="dram", bufs=1, space="DRAM")

    # ---- persistent SBUF constants ----
    ident = const.tile([128, 128], f32, tag="ident", name="ident")
    make_identity(nc, ident[:])
    identb = const.tile([128, 128], bf16, tag="identb", name="identb")
    make_identity(nc, identb[:])
    i16 = const.tile([16, 16], bf16, tag="i16", name="i16")
    make_identity(nc, i16[:])
    ones1 = const.tile([1, 16], bf16, tag="ones1", name="ones1")
    nc.gpsimd.memset(ones1[:], 1.0)
    onesr = const.tile([1, 128], bf16, tag="onesr", name="onesr")
    nc.gpsimd.memset(onesr[:], 1.0)
    biasf = const.tile([1, 2048], bf16, tag="biasf", name="biasf")
    biasb = const.tile([1, 2048], bf16, tag="biasb", name="biasb")
    bias_stat = const.tile([4, 16], bf16, tag="bias_stat", name="bias_stat")
    nc.gpsimd.memset(bias_stat[:], 0.0)
    nc.gpsimd.memset(bias_stat[0:1, :], 1.0)
    junkW = const.tile([128, 512], bf16, tag="junkW", name="junkW")
    nc.gpsimd.memset(junkW[:], 0.0)

    def tr(out_ap, in_ap, pin):
        nc.tensor.transpose(out_ap, in_ap, ident[0:pin, 0:pin])

    idx_sb = const.tile([128, 32], i32, tag="idx", name="idx")
    nc.sync.dma_start(idx_sb[:], idx_d[:])

    wih_sb = [const.tile([128, G4], bf16, tag=f"wih{k}", name=f"wih{k}") for k in range(3)]
    bwih_sb = [const.tile([128, G4], bf16, tag=f"bwih{k}", name=f"bwih{k}") for k in range(3)]
    for k in range(2):
        nc.sync.dma_start(wih_sb[k][:], wihT_d[128 * k:128 * (k + 1), :])
        nc.sync.dma_start(bwih_sb[k][:], bwihT_d[128 * k:128 * (k + 1), :])
    nc.sync.dma_start(wih_sb[2][0:44, :], wihT_d[256:300, :])
    nc.sync.dma_start(bwih_sb[2][0:44, :], bwihT_d[256:300, :])
    nc.sync.dma_start(biasf[:], wihT_d[300:301, :])
    nc.sync.dma_start(biasb[:], bwihT_d[300:301, :])

    whh_sb = [const.tile([128, G4], bf16, tag=f"whh{k}", name=f"whh{k}") for k in range(4)]
    for k in range(4):
        nc.sync.dma_start(whh_sb[k][:], whhT_d[128 * k:128 * (k + 1), :])

    dwhh_sb = [const.tile([128, G3], bf16, tag=f"dwhh{k}", name=f"dwhh{k}") for k in range(9)]
    for k in range(8):
        nc.sync.dma_start(dwhh_sb[k][:], dwhhT_d[128 * k:128 * (k + 1), :])
    nc.sync.dma_start(dwhh_sb[8][0:4, :], dwhhT_d[1024:1028, :])

    dwih_sb = [const.tile([128, G3], bf16, tag=f"dwih{k}", name=f"dwih{k}") for k in range(5)]
    for k in range(4):
        nc.sync.dma_start(dwih_sb[k][:], dwihT_d[128 * k:128 * (k + 1), :])
    nc.sync.dma_start(dwih_sb[4][0:4, :], dwihT_d[512:516, :])

    cls_sb = [const.tile([128, 2], bf16, tag=f"cls{k}", name=f"cls{k}") for k in range(9)]
    for k in range(8):
        nc.sync.dma_start(cls_sb[k][:], clsT_d[128 * k:128 * (k + 1), :])
    nc.sync.dma_start(cls_sb[8][0:4, :], clsT_d[1024:1028, :])

    bH = const.tile([128, 64], bf16, tag="bH", name="bH")  # backward-cell h' (2h)

    # ---- pools (released LIFO; pH outlives the scan into the decoder) ----
    pH = pool(name="pH", bufs=2)
    pA = pool(name="pA", bufs=3)       # gather / tanh / embT tiles
    pXP = pool(name="pXP", bufs=3)     # xp [128, G4] bf16 per 8-step window
    pST = pool(name="pST", bufs=6)     # per-step [16, G4] stage (partition 0)
    pT = pool(name="pT", bufs=2)
    pZ = pool(name="pZ", bufs=2)
    pW = pool(name="pW", bufs=2)
    pAps2 = pool(name="pAps2", bufs=1, space="PSUM")   # A transposes (1 bank)
    pApsx = pool(name="pApsx", bufs=2, space="PSUM")   # per-nb xproj psum (2 banks)
    pPS = pool(name="pPS", bufs=2, space="PSUM")       # psg / pstT
    pJ = pool(name="pJ", bufs=1, space="PSUM")         # junk bank for warmers

    emb_sb = {}

    def emit_gather(m):
        g_t = pA.tile([128, 304], f32, tag="gath", name="gath")
        nc.gpsimd.indirect_dma_start(
            out=g_t[:, 0:E],
            out_offset=None,
            in_=embedW_d[:],
            in_offset=bass.IndirectOffsetOnAxis(ap=idx_sb[:, m:m + 1], axis=0),
        )
        return g_t

    gath = {}

    A_th = {}

    def emit_A_tanh(m):
        # deferred to the post-vec slot: off the step-critical scalar queue
        th = pA.tile([128, 304], f32, tag="th", name="th")
        nc.scalar.activation(th[:, 0:E], gath[m][:, 0:E], Tanh)
        A_th[m] = th

    def emit_A_tr(m):
        th = A_th.pop(m)
        pst = pAps2.tile([128, 384], f32, tag="pst", name="pst")
        tr(pst[0:128, 0:128], th[:, 0:128], 128)
        tr(pst[0:128, 128:256], th[:, 128:256], 128)
        tr(pst[0:44, 256:384], th[:, 256:300], 128)
        embT_a = pA.tile([128, 256], bf16, tag="embTa", name="embTa")
        nc.vector.tensor_copy(embT_a[:], pst[:, 0:256])
        embT_b = pA.tile([48, 128], bf16, tag="embTb", name="embTb")
        nc.vector.tensor_copy(embT_b[0:44, :], pst[0:44, 256:384])
        emb_sb[m] = (embT_a, embT_b)

    def emit_A_nb_mm(m, nb):
        embT_a, embT_b = emb_sb[m]
        nsl = slice(512 * nb, 512 * (nb + 1))
        psx = pApsx.tile([128, 512], f32, tag="psx", name="psx")
        nc.tensor.matmul(psx[:], embT_a[:, 0:128], wih_sb[0][:, nsl],
                         start=True, stop=False)
        nc.tensor.matmul(psx[:], embT_a[:, 128:256], wih_sb[1][:, nsl],
                         start=False, stop=False)
        nc.tensor.matmul(psx[:], embT_b[0:44, :], wih_sb[2][0:44, nsl],
                         start=False, stop=False)
        nc.tensor.matmul(psx[:], onesr[:, 0:128], biasf[0:1, nsl],
                         start=False, stop=True)
        return psx

    def emit_A_nb_copy(m, nb, psx):
        nsl = slice(512 * nb, 512 * (nb + 1))
        if nb % 2 == 0:
            nc.vector.tensor_copy(xp_tiles[m][:, nsl], psx[:])
        else:
            nc.scalar.activation(xp_tiles[m][:, nsl], psx[:], Ident)

    def emit_junk(n, jpool=None):
        # dense warm-keeper burst: one accumulation group so the MMs stream
        # back-to-back (start=True per-MM would serialize on bank drain)
        jp = (jpool or pJ).tile([128, 512], f32, tag="junk", name="junk")
        for k in range(n):
            nc.tensor.matmul(jp[:], identb[:], junkW[:],
                             start=(k == 0), stop=(k == n - 1))

    # ======== lead-in: A(0), A(1) full + backward cell ========
    xp_tiles = {}
    for m in (0, 1, 2):
        gath[m] = emit_gather(m)
    for m in (0, 1):
        xp_tiles[m] = pXP.tile([128, G4], bf16, tag="xp", name="xp")
        emit_A_tanh(m)
        emit_A_tr(m)
        for nb in range(4):
            psx = emit_A_nb_mm(m, nb)
            emit_A_nb_copy(m, nb, psx)

    # backward LSTM single cell on x_0 (h=c=0), reusing pPS banks
    emb0_a, emb0_b = emb_sb[0]
    bps = pPS.tile([128, 512], f32, tag="psg", name="bps")
    for j in range(4):
        ns = slice(512 * j, 512 * (j + 1))
        o = bps[32 * j:32 * j + 16, :]
        tp = (0, 32 * j)
        nc.tensor.matmul(o, emb0_a[:, 0:16], bwih_sb[0][:, ns],
                         start=True, stop=False, tile_position=tp)
        nc.tensor.matmul(o, emb0_a[:, 128:144], bwih_sb[1][:, ns],
                         start=False, stop=False, tile_position=tp)
        nc.tensor.matmul(o, emb0_b[0:44, 0:16], bwih_sb[2][0:44, ns],
                         start=False, stop=False, tile_position=tp)
        nc.tensor.matmul(o, onesr[:, 0:16], biasb[0:1, ns],
                         start=False, stop=True, tile_position=tp)
    bT = pA.tile([128, 512], f32, tag="bT", name="bT")
    nc.scalar.activation(bT[0:112, :], bps[0:112, :], Tanh)
    bpt = pPS.tile([128, 448], f32, tag="pstT", name="bpt")
    for k in range(4):
        tr(bpt[:, 112 * k:112 * (k + 1)], bT[0:112, 128 * k:128 * (k + 1)], 112)
    bv = bpt[:].rearrange("p (c w) -> p c w", w=112)
    btip = pA.tile([128, 64], f32, tag="btip", name="btip")
    nc.vector.tensor_scalar_add(
        out=btip[:].rearrange("p (c w) -> p c w", w=16),
        in0=bv[:, :, 0:16], scalar1=1.0)
    bzv = pA.tile([128, 64], f32, tag="bzv", name="bzv")
    nc.vector.tensor_tensor(
        out=bzv[:].rearrange("p (c w) -> p c w", w=16),
        in0=btip[:].rearrange("p (c w) -> p c w", w=16),
        in1=bv[:, :, 64:80], op=MUL)
    btc = pA.tile([128, 64], f32, tag="btc", name="btc")
    nc.scalar.activation(btc[:], bzv[:], Tanh, scale=0.5)
    nc.vector.scalar_tensor_tensor(
        out=bH[:].rearrange("p (c w) -> p c w", w=16),
        in0=bv[:, :, 96:112], scalar=1.0,
        in1=btc[:].rearrange("p (c w) -> p c w", w=16),
        op0=ADD, op1=MUL)

    # ======== forward scan, 256 steps, phase A interleaved ========
    z_prev = [pZ.tile([128, 32], f32, tag=f"z{h}", name=f"z{h}") for h in range(2)]
    H_prev = [pH.tile([128, 32], bf16, tag=f"H{h}", name=f"H{h}") for h in range(2)]
    for h in range(2):
        nc.vector.memset(z_prev[h][:], 0.0)
        nc.vector.memset(H_prev[h][:], 0.0)
    emit_junk(10)

    def emit_stage(t):
        st = pST.tile([16, G4], bf16, tag="st", name="st")
        sub = t % 8
        nc.sync.dma_start(st[:], xp_tiles[t // 8][16 * sub:16 * sub + 16, :])
        return st

    stage = {t: emit_stage(t) for t in range(4)}

    def emit_inject(t, psg):
        xp_src = stage.pop(t)
        for j in range(4):
            nc.tensor.matmul(
                psg[32 * j:32 * j + 16, :], i16[:],
                xp_src[:, 512 * j:512 * (j + 1)],
                start=True, stop=False, tile_position=(0, 32 * j))

    psg_cur = pPS.tile([128, 512], f32, tag="psg", name="psg")
    emit_inject(0, psg_cur)

    for t in range(S):
        m = t // 8
        sub = t % 8
        has_A = m + 2 <= 31
        a_mm = has_A and 3 <= sub <= 6
        if t + 4 < S:
            stage[t + 4] = emit_stage(t + 4)

        # ---- gate matmuls for step t (xp inject already accumulated) ----
        for kc in range(4):
            hh = H_prev[kc // 2]
            col = 16 * (kc % 2)
            for j in range(4):
                nc.tensor.matmul(
                    psg_cur[32 * j:32 * j + 16, :], hh[:, col:col + 16],
                    whh_sb[kc][:, 512 * j:512 * (j + 1)],
                    start=False, stop=(kc == 3), tile_position=(0, 32 * j))
        emit_junk(DN)

        # ---- nonlinearity + transposes, split in hidden halves ----
        T_t = pT.tile([128, 512], f32, tag="T", name="T")
        nc.scalar.activation(T_t[0:112, 0:256], psg_cur[0:112, 0:256], Tanh)
        nc.scalar.activation(T_t[0:112, 256:512], psg_cur[0:112, 256:512], Tanh)
        pstT = pPS.tile([128, 448], f32, tag="pstT", name="pstT")
        for k in range(4):
            tr(pstT[:, 112 * k:112 * (k + 1)], T_t[0:112, 128 * k:128 * (k + 1)], 112)

        # ---- pre-issue next step's inject; fill PE with A matmuls / junk ----
        psg_nxt = None
        if t + 1 < S:
            psg_nxt = pPS.tile([128, 512], f32, tag="psg", name="psg")
            emit_inject(t + 1, psg_nxt)
        a_psx = None
        if has_A and a_mm:
            a_psx = emit_A_nb_mm(m + 2, sub - 3)
        if has_A and sub == 2:
            emit_A_tr(m + 2)  # PE transposes; A-tanh ran in sub1's tail
        emit_junk(DN2 - 2 if a_mm else DN2)

        # ---- vec chains per hidden half ----
        Tv = pstT[:].rearrange("p (c w) -> p c w", w=112)
        z_new = [pZ.tile([128, 32], f32, tag=f"z{h}", name=f"z{h}") for h in range(2)]
        H_new = [pH.tile([128, 32], bf16, tag=f"H{h}", name=f"H{h}") for h in range(2)]
        tc_ts = []
        for h in range(2):
            cs = slice(2 * h, 2 * h + 2)
            ti, tf = Tv[:, cs, 0:16], Tv[:, cs, 32:48]
            tg = Tv[:, cs, 64:80]
            zpv = z_prev[h][:].rearrange("p (c w) -> p c w", w=16)
            a_t = pW.tile([128, 32], f32, tag=f"a{h}", name=f"a{h}")
            av = a_t[:].rearrange("p (c w) -> p c w", w=16)
            nc.vector.scalar_tensor_tensor(out=av, in0=tf, scalar=1.0, in1=zpv,
                                           op0=ADD, op1=MUL)
            tip = pW.tile([128, 32], f32, tag=f"tip{h}", name=f"tip{h}")
            tipv = tip[:].rearrange("p (c w) -> p c w", w=16)
            nc.vector.tensor_scalar_add(out=tipv, in0=ti, scalar1=1.0)
            v_t = pW.tile([128, 32], f32, tag=f"v{h}", name=f"v{h}")
            vv = v_t[:].rearrange("p (c w) -> p c w", w=16)
            nc.vector.tensor_tensor(out=vv, in0=tipv, in1=tg, op=MUL)
            nc.vector.scalar_tensor_tensor(out=z_new[h][:], in0=a_t[:], scalar=0.5,
                                           in1=v_t[:], op0=MUL, op1=ADD)
            tc_t = pW.tile([128, 32], f32, tag=f"tc{h}", name=f"tc{h}")
            nc.scalar.activation(tc_t[:], z_new[h][:], Tanh, scale=0.5)
            tc_ts.append(tc_t)
        for h in range(2):
            cs = slice(2 * h, 2 * h + 2)
            to = Tv[:, cs, 96:112]
            nc.vector.scalar_tensor_tensor(
                out=H_new[h][:].rearrange("p (c w) -> p c w", w=16),
                in0=to, scalar=1.0,
                in1=tc_ts[h][:].rearrange("p (c w) -> p c w", w=16),
                op0=ADD, op1=MUL)

        # ---- deferred off-chain A work (runs while next step computes) ----
        if has_A:
            if sub == 0 and m + 3 <= 31:
                gath[m + 3] = emit_gather(m + 3)
            elif sub == 1:
                xp_tiles[m + 2] = pXP.tile([128, G4], bf16, tag="xp", name="xp")
                emit_A_tanh(m + 2)
            elif a_mm:
                emit_A_nb_copy(m + 2, sub - 3, a_psx)

        z_prev, H_prev = z_new, H_new
        if psg_nxt is not None:
            psg_cur = psg_nxt

    pJ.release()
    pPS.release()
    pApsx.release()
    pAps2.release()
    pW.release()
    pZ.release()
    pT.release()
    pST.release()
    pXP.release()
    pA.release()

    # ======== decoder (6 GRU steps + logits + log_softmax) ========
    pD = pool(name="pD", bufs=1)
    pDgi = pool(name="pDgi", bufs=1, space="PSUM")

    ce_t = pD.tile([NCLS, H], f32, tag="ce", name="ce")
    nc.sync.dma_start(ce_t[:], ecw_d[:])
    ce2 = pD.tile([NCLS, H], f32, tag="ce2", name="ce2")
    nc.scalar.activation(ce2[:], ce_t[:], Tanh)
    psc = pDgi.tile([128, 24], f32, tag="psc", name="psc")
    for k in range(4):
        tr(psc[:, 6 * k:6 * (k + 1)], ce2[0:NCLS, 128 * k:128 * (k + 1)], NCLS)
    ceT = pD.tile([128, 24], bf16, tag="ceT", name="ceT")
    nc.vector.tensor_copy(ceT[:], psc[:])

    psgi = pDgi.tile([NCLS, G3], f32, tag="psgi", name="psgi")
    for ng in range(6):
        ns = slice(512 * ng, 512 * (ng + 1))
        for kc in range(4):
            nc.tensor.matmul(psgi[:, ns], ceT[:, 6 * kc:6 * (kc + 1)],
                             dwih_sb[kc][:, ns], start=(kc == 0), stop=False)
        nc.tensor.matmul(psgi[:, ns], bias_stat[0:4, 0:NCLS],
                         dwih_sb[4][0:4, ns], start=False, stop=True)
    gi_sb = pD.tile([NCLS, G3], bf16, tag="gi", name="gi")
    nc.scalar.activation(gi_sb[:], psgi[:], Ident)

    # transposed gi_n (per-partition bias for the n-gate)
    psgT = pDgi.tile([128, 48], bf16, tag="psgT", name="psgT")
    for gc in range(8):
        nc.tensor.transpose(psgT[:, 6 * gc:6 * (gc + 1)],
                            gi_sb[0:NCLS, 2048 + 128 * gc:2048 + 128 * (gc + 1)],
                            identb[0:NCLS, 0:NCLS])
    giT = pD.tile([128, 48], f32, tag="giT", name="giT")
    nc.vector.tensor_copy(giT[:], psgT[:])

    # per-class gi rows at partition 0, via DRAM bounce (avoids recompute)
    gi_dram = dramp.tile([NCLS, G3], bf16, tag="gid", name="gid")
    nc.sync.dma_start(gi_dram[:], gi_sb[:])
    gi_row = pD.tile([1, NCLS * G3], bf16, tag="girow", name="girow")
    for c in range(NCLS):
        nc.sync.dma_start(gi_row[0:1, G3 * c:G3 * (c + 1)],
                          gi_dram[c:c + 1, :])
    pDgi.release()
    pDps = pool(name="pDps", bufs=1, space="PSUM")
    pJ2 = pool(name="pJ2", bufs=1, space="PSUM")

    Hd = pD.tile([128, 128], bf16, tag="Hd", name="Hd")
    nc.vector.tensor_scalar_mul(Hd[:, 0:32], H_prev[0][:], 0.5)
    nc.vector.tensor_scalar_mul(Hd[:, 32:64], H_prev[1][:], 0.5)
    nc.vector.tensor_scalar_mul(Hd[:, 64:128], bH[:], 0.5)

    l_all = pD.tile([16, 12], f32, tag="lall", name="lall")

    for c in range(NCLS):
        psd0 = pDps.tile([128, 512], f32, tag="psd0", name="psd0")
        psd1 = pDps.tile([128, 512], f32, tag="psd1", name="psd1")
        for kc in range(8):
            lh = Hd[:, 16 * kc:16 * (kc + 1)]
            for ng in range(6):
                ps, j = (psd0, ng) if ng < 4 else (psd1, ng - 4)
                nc.tensor.matmul(
                    ps[32 * j:32 * j + 16, :], lh,
                    dwhh_sb[kc][:, 512 * ng:512 * (ng + 1)],
                    start=(kc == 0), stop=False, tile_position=(0, 32 * j))
        for ng in range(6):
            ps, j = (psd0, ng) if ng < 4 else (psd1, ng - 4)
            nc.tensor.matmul(
                ps[32 * j:32 * j + 16, :], bias_stat[0:4, :],
                dwhh_sb[8][0:4, 512 * ng:512 * (ng + 1)],
                start=False, stop=(ng >= 4), tile_position=(0, 32 * j))
        for ng in range(4):
            nc.tensor.matmul(
                psd0[32 * ng:32 * ng + 16, :], ones1[:],
                gi_row[0:1, G3 * c + 512 * ng:G3 * c + 512 * (ng + 1)],
                start=False, stop=True, tile_position=(0, 32 * ng))

        Trz = pD.tile([128, 512], f32, tag="Trz", name="Trz")
        nc.scalar.activation(Trz[0:112, :], psd0[0:112, :], Tanh)
        pstz = pDps.tile([128, 448], f32, tag="pstz", name="pstz")
        for k in range(4):
            tr(pstz[:, 112 * k:112 * (k + 1)], Trz[0:112, 128 * k:128 * (k + 1)], 112)
        hn_sb = pD.tile([48, 512], f32, tag="hn", name="hn")
        nc.vector.tensor_copy(hn_sb[:], psd1[0:48, :])
        psn = pDps.tile([128, 192], f32, tag="psn", name="psn")
        for k in range(4):
            tr(psn[:, 48 * k:48 * (k + 1)], hn_sb[0:48, 128 * k:128 * (k + 1)], 48)
        emit_junk(10, pJ2)

        zv = pstz[:].rearrange("p (c w) -> p c w", w=112)
        nv = psn[:].rearrange("p (c w) -> p c w", w=48)
        trp = pD.tile([128, 128], f32, tag="trp", name="trp")
        trpv = trp[:].rearrange("p (g w) -> p g w", w=16)
        sn_t = pD.tile([128, 128], f32, tag="sn", name="sn")
        snv = sn_t[:].rearrange("p (g w) -> p g w", w=16)
        nT = pD.tile([128, 128], f32, tag="nT", name="nT")
        for s in range(2):
            nc.vector.tensor_scalar_add(
                out=trpv[:, 4 * s:4 * s + 4, :],
                in0=zv[:, :, 32 * s:32 * s + 16], scalar1=1.0)
            nc.vector.tensor_tensor(
                out=snv[:, 4 * s:4 * s + 4, :],
                in0=trpv[:, 4 * s:4 * s + 4, :],
                in1=nv[:, :, 32 * s:32 * s + 16], op=MUL)
        nTv = nT[:].rearrange("p (g w) -> p g w", w=16)
        for g in range(8):
            nc.scalar.activation(nTv[:, g, :], snv[:, g, :], Tanh, scale=0.5,
                                 bias=giT[:, 6 * g + c:6 * g + c + 1])
        d_t = pD.tile([128, 128], f32, tag="dt", name="dt")
        nc.vector.tensor_tensor(out=d_t[:], in0=Hd[:], in1=nT[:], op=SUB)
        e_t = pD.tile([128, 128], f32, tag="et", name="et")
        ev = e_t[:].rearrange("p (g w) -> p g w", w=16)
        dv = d_t[:].rearrange("p (g w) -> p g w", w=16)
        for s in range(2):
            nc.vector.scalar_tensor_tensor(
                out=ev[:, 4 * s:4 * s + 4, :],
                in0=zv[:, :, 64 + 32 * s:80 + 32 * s], scalar=1.0,
                in1=dv[:, 4 * s:4 * s + 4, :], op0=ADD, op1=MUL)
        hn2 = pD.tile([128, 128], f32, tag="hn2", name="hn2")
        nc.vector.scalar_tensor_tensor(out=hn2[:], in0=e_t[:], scalar=0.5,
                                       in1=nT[:], op0=MUL, op1=ADD)
        Hd_new = pD.tile([128, 128], bf16, tag="Hd", name="Hd")
        nc.scalar.activation(Hd_new[:], hn2[:], Tanh)

        psl = pDps.tile([16, 2], f32, tag="psl", name="psl")
        for kc in range(8):
            nc.tensor.matmul(psl[:], Hd_new[:, 16 * kc:16 * (kc + 1)],
                             cls_sb[kc][:, 0:2], start=(kc == 0), stop=False)
        nc.tensor.matmul(psl[:], bias_stat[0:4, :], cls_sb[8][0:4, 0:2],
                         start=False, stop=True)
        nc.vector.tensor_copy(l_all[:, 2 * c:2 * c + 2], psl[:])
        Hd = Hd_new

    la = l_all[:].rearrange("p (c t) -> p c t", t=2)
    mx = pD.tile([16, 6], f32, tag="mx", name="mx")
    nc.vector.tensor_tensor(out=mx[:].rearrange("p (c o) -> p c o", o=1),
                            in0=la[:, :, 0:1], in1=la[:, :, 1:2], op=MAX)
    d0 = pD.tile([16, 12], f32, tag="d0", name="d0")
    d0v = d0[:].rearrange("p (c t) -> p c t", t=2)
    mxb = mx[:].rearrange("p (c o) -> p c o", o=1).to_broadcast([16, 6, 2])
    nc.vector.tensor_tensor(out=d0v, in0=la, in1=mxb, op=SUB)
    ex = pD.tile([16, 12], f32, tag="ex", name="ex")
    nc.scalar.activation(ex[:], d0[:], Exp)
    se = pD.tile([16, 6], f32, tag="se", name="se")
    nc.vector.tensor_reduce(out=se[:].rearrange("p (c o) -> p c o", o=1),
                            in_=ex[:].rearrange("p (c t) -> p c t", t=2),
                            op=ADD, axis=mybir.AxisListType.X)
    ls = pD.tile([16, 6], f32, tag="ls", name="ls")
    nc.scalar.activation(ls[:], se[:], Ln)
    ov = pD.tile([16, 12], f32, tag="ov", name="ov")
    lsb = ls[:].rearrange("p (c o) -> p c o", o=1).to_broadcast([16, 6, 2])
    nc.vector.tensor_tensor(out=ov[:].rearrange("p (c t) -> p c t", t=2),
                            in0=d0v, in1=lsb, op=SUB)
    nc.sync.dma_start(out_d[:].rearrange("c b t -> b c t"),
                      ov[:].rearrange("p (c t) -> p c t", t=2))

    pJ2.release()
    pDps.release()
    pD.release()
    pH.release()
    dramp.release()
    const.release()


def _prep_inputs(seq, classes, embed_W, embed_class_W, f_Wih, f_Whh, f_b,
                 b_Wih, b_Whh, b_b, d_Wih, d_Whh, d_bih, d_bhh, cls_W, cls_b):
    seq = np.asarray(seq)
    s4 = np.concatenate([np.full(H, 0.5), np.full(H, 0.5), np.ones(H),
                         np.full(H, 0.5)]).astype(np.float32)
    s3 = np.concatenate([np.full(H2, 0.5), np.full(H2, 0.5),
                         np.ones(H2)]).astype(np.float32)

    def padrows(a, rows):
        out = np.zeros((rows, a.shape[1]), np.float32)
        out[:a.shape[0]] = a
        return out

    wihT = padrows(np.concatenate(
        [(f_Wih * s4[:, None]).T, (f_b * s4)[None, :]], axis=0), 304)
    bwihT = padrows(np.concatenate(
        [(b_Wih * s4[:, None]).T, (b_b * s4)[None, :]], axis=0), 304)
    whhT = ((f_Whh * s4[:, None]) * 0.5).T.astype(np.float32)
    dwhhT = padrows(np.concatenate(
        [(d_Whh * s3[:, None]).T, (d_bhh * s3)[None, :]], axis=0), 1028)
    dwihT = padrows(np.concatenate(
        [(d_Wih * s3[:, None]).T, (d_bih * s3)[None, :]], axis=0), 516)
    clsT = padrows(np.concatenate(
        [np.asarray(cls_W, np.float32).T, np.asarray(cls_b, np.float32)[None, :]],
        axis=0), 1028)
    ecw = np.asarray(embed_class_W, np.float32)[np.asarray(classes)]

    shared = {
        "embedW": np.ascontiguousarray(np.asarray(embed_W, np.float32)),
        "wihT": _bf(wihT), "bwihT": _bf(bwihT), "whhT": _bf(whhT),
        "dwhhT": _bf(dwhhT), "dwihT": _bf(dwihT),
        "ecw": np.ascontiguousarray(ecw),
        "clsT": _bf(clsT),
    }
    in_maps = []
    for c in range(NC):
        tok = np.asarray(seq[BL * c:BL * (c + 1), :], np.int32)  # [16, 256]
        idx = np.ascontiguousarray(
            tok.T.reshape(S * BL).reshape(32, 128).T.astype(np.int32))
        m = dict(shared)
        m["idx"] = idx
        in_maps.append(m)
    return in_maps


def kernel(**inputs):
    if "nc" not in _cache:
        _cache["nc"] = _build_program()
    nc = _cache["nc"]
    in_maps = _prep_inputs(**inputs)
    import os
    trace = bool(int(os.environ.get("BK_TRACE", "0")))
    res = run_bass_kernel_spmd(nc, in_maps, core_ids=list(range(NC)),
                               trace=trace)
    _cache["last_result"] = res
    outs = [res.results[c]["out"] for c in range(NC)]
    return np.concatenate(outs, axis=1).astype(np.float32)


# revision 24
# speedup vs baseline: 1.2159x; 1.1354x over previous
"""Trainium2 Bass kernel for nn_BiLSTMDecoderModel.

Strategy (8 NeuronCores, data-parallel over batch, B=128 -> 16 rows/core):
  * backward LSTM: only b_hs[0] is consumed downstream == ONE cell step on x_0.
  * forward LSTM: 256-step scan, h-stationary recurrent matmul with 4-way PE
    column tiling; xp injected into PSUM first (identity-stationary matmul).
  * gate nonlinearities collapse to tanh via sigmoid(x)=(tanh(x/2)+1)/2 with
    scale factors folded into host-preprocessed weights (carried z=2c, h'=2h).
  * phase A (gather + tanh + transpose + input projection) is INTERLEAVED with
    the scan (per-nb parts spread across each 8-step window) and feeds the scan
    directly from SBUF; dummy matmuls fill remaining PE idle so the HAM clock
    gate stays at K=8/8 (2.4 GHz) instead of 1.2 GHz.
  * decoder: gi GEMM computed once; per-class rows obtained via SBUF->SBUF DMA
    row moves instead of recomputing the GEMM per class.
"""

import sys

sys.path.insert(0, "/opt/trn_rl_repo")

import numpy as np
import ml_dtypes

import concourse.bass as bass
import concourse.mybir as mybir
import concourse.tile as tile
from concourse import bacc
from concourse.bass_utils import run_bass_kernel_spmd
from concourse.masks import make_identity

V, E, H, NCLS = 100000, 300, 512, 6
B, S = 128, 256
NC = 8
BL = B // NC  # 16
G4 = 4 * H  # 2048
G3 = 3 * 2 * H  # 3072
H2 = 2 * H  # 1024

DN = 3    # full-size warm-keeper matmuls after the per-step MM block
DN2 = 5   # full-size warm-keepers after the transposes (cover the vec chain)

f32 = mybir.dt.float32
bf16 = mybir.dt.bfloat16
i32 = mybir.dt.int32
Tanh = mybir.ActivationFunctionType.Tanh
Exp = mybir.ActivationFunctionType.Exp
Ln = mybir.ActivationFunctionType.Ln
Ident = mybir.ActivationFunctionType.Identity
ADD = mybir.AluOpType.add
SUB = mybir.AluOpType.subtract
MUL = mybir.AluOpType.mult
MAX = mybir.AluOpType.max

_cache = {}


def _bf(x):
    return np.ascontiguousarray(x.astype(ml_dtypes.bfloat16))


def _build_program():
    nc = bacc.Bacc(
        "TRN2", target_bir_lowering=False, debug=False, enable_asserts=False,
        num_devices=NC,
    )
    embedW_d = nc.dram_tensor("embedW", [V, E], f32, kind="ExternalInput").ap()
    idx_d = nc.dram_tensor("idx", [128, 32], i32, kind="ExternalInput").ap()
    wihT_d = nc.dram_tensor("wihT", [304, G4], bf16, kind="ExternalInput").ap()
    bwihT_d = nc.dram_tensor("bwihT", [304, G4], bf16, kind="ExternalInput").ap()
    whhT_d = nc.dram_tensor("whhT", [H, G4], bf16, kind="ExternalInput").ap()
    dwhhT_d = nc.dram_tensor("dwhhT", [1028, G3], bf16, kind="ExternalInput").ap()
    dwihT_d = nc.dram_tensor("dwihT", [516, G3], bf16, kind="ExternalInput").ap()
    ecw_d = nc.dram_tensor("ecw", [NCLS, H], f32, kind="ExternalInput").ap()
    clsT_d = nc.dram_tensor("clsT", [1028, 2], bf16, kind="ExternalInput").ap()
    out_d = nc.dram_tensor("out", [NCLS, BL, 2], f32, kind="ExternalOutput").ap()

    with tile.TileContext(nc) as tc:
        _emit(nc, tc, embedW_d, idx_d, wihT_d, bwihT_d, whhT_d, dwhhT_d,
              dwihT_d, ecw_d, clsT_d, out_d)
    nc.compile()
    return nc


def _emit(nc, tc, embedW_d, idx_d, wihT_d, bwihT_d, whhT_d, dwhhT_d, dwihT_d,
          ecw_d, clsT_d, out_d):
    def pool(**kw):
        return tc.alloc_tile_pool(**kw)

    const = pool(name="const", bufs=1)
    dramp = pool(name="dram", bufs=1, space="DRAM")

    # ---- persistent SBUF constants ----
    ident = const.tile([128, 128], f32, tag="ident", name="ident")
    make_identity(nc, ident[:])
    identb = const.tile([128, 128], bf16, tag="identb", name="identb")
    make_identity(nc, identb[:])
    i16 = const.tile([16, 16], bf16, tag="i16", name="i16")
    make_identity(nc, i16[:])
    ones1 = const.tile([1, 16], bf16, tag="ones1", name="ones1")
    nc.gpsimd.memset(ones1[:], 1.0)
    onesr = const.tile([1, 128], bf16, tag="onesr", name="onesr")
    nc.gpsimd.memset(onesr[:], 1.0)
    biasf = const.tile([1, 2048], bf16, tag="biasf", name="biasf")
    biasb = const.tile([1, 2048], bf16, tag="biasb", name="biasb")
    bias_stat = const.tile([4, 16], bf16, tag="bias_stat", name="bias_stat")
    nc.gpsimd.memset(bias_stat[:], 0.0)
    nc.gpsimd.memset(bias_stat[0:1, :], 1.0)
    junkW = const.tile([128, 512], bf16, tag="junkW", name="junkW")
    nc.gpsimd.memset(junkW[:], 0.0)

    def tr(out_ap, in_ap, pin):
        nc.tensor.transpose(out_ap, in_ap, ident[0:pin, 0:pin])

    idx_sb = const.tile([128, 32], i32, tag="idx", name="idx")
    nc.sync.dma_start(idx_sb[:], idx_d[:])

    wih_sb = [const.tile([128, G4], bf16, tag=f"wih{k}", name=f"wih{k}") for k in range(3)]
    bwih_sb = [const.tile([128, G4], bf16, tag=f"bwih{k}", name=f"bwih{k}") for k in range(3)]
    for k in range(2):
        nc.sync.dma_start(wih_sb[k][:], wihT_d[128 * k:128 * (k + 1), :])
        nc.sync.dma_start(bwih_sb[k][:], bwihT_d[128 * k:128 * (k + 1), :])
    nc.sync.dma_start(wih_sb[2][0:44, :], wihT_d[256:300, :])
    nc.sync.dma_start(bwih_sb[2][0:44, :], bwihT_d[256:300, :])
    nc.sync.dma_start(biasf[:], wihT_d[300:301, :])
    nc.sync.dma_start(biasb[:], bwihT_d[300:301, :])

    whh_sb = [const.tile([128, G4], bf16, tag=f"whh{k}", name=f"whh{k}") for k in range(4)]
    for k in range(4):
        nc.sync.dma_start(whh_sb[k][:], whhT_d[128 * k:128 * (k + 1), :])

    dwhh_sb = [const.tile([128, G3], bf16, tag=f"dwhh{k}", name=f"dwhh{k}") for k in range(9)]
    for k in range(8):
        nc.sync.dma_start(dwhh_sb[k][:], dwhhT_d[128 * k:128 * (k + 1), :])
    nc.sync.dma_start(dwhh_sb[8][0:4, :], dwhhT_d[1024:1028, :])

    dwih_sb = [const.tile([128, G3], bf16, tag=f"dwih{k}", name=f"dwih{k}") for k in range(5)]
    for k in range(4):
        nc.sync.dma_start(dwih_sb[k][:], dwihT_d[128 * k:128 * (k + 1), :])
    nc.sync.dma_start(dwih_sb[4][0:4, :], dwihT_d[512:516, :])

    cls_sb = [const.tile([128, 2], bf16, tag=f"cls{k}", name=f"cls{k}") for k in range(9)]
    for k in range(8):
        nc.sync.dma_start(cls_sb[k][:], clsT_d[128 * k:128 * (k + 1), :])
    nc.sync.dma_start(cls_sb[8][0:4, :], clsT_d[1024:1028, :])

    bH = const.tile([128, 64], bf16, tag="bH", name="bH")  # backward-cell h' (2h)

    # ---- pools (released LIFO; pH outlives the scan into the decoder) ----
    pH = pool(name="pH", bufs=2)
    pA = pool(name="pA", bufs=3)       # gather / tanh / embT tiles
    pXP = pool(name="pXP", bufs=3)     # xp [128, G4] bf16 per 8-step window
    pST = pool(name="pST", bufs=6)     # per-step [16, G4] stage (partition 0)
    pT = pool(name="pT", bufs=2)
    pZ = pool(name="pZ", bufs=2)
    pW = pool(name="pW", bufs=2)
    pAps2 = pool(name="pAps2", bufs=1, space="PSUM")   # A transposes (1 bank)
    pApsx = pool(name="pApsx", bufs=2, space="PSUM")   # per-nb xproj psum (2 banks)
    pPS = pool(name="pPS", bufs=2, space="PSUM")       # psg / pstT
    pJ = pool(name="pJ", bufs=1, space="PSUM")         # junk bank for warmers

    emb_sb = {}

    def emit_gather(m):
        g_t = pA.tile([128, 304], f32, tag="gath", name="gath")
        nc.gpsimd.indirect_dma_start(
            out=g_t[:, 0:E],
            out_offset=None,
            in_=embedW_d[:],
            in_offset=bass.IndirectOffsetOnAxis(ap=idx_sb[:, m:m + 1], axis=0),
        )
        return g_t

    gath = {}

    A_th = {}

    def emit_A_tanh(m):
        # deferred to the post-vec slot: off the step-critical scalar queue
        th = pA.tile([128, 304], f32, tag="th", name="th")
        nc.scalar.activation(th[:, 0:E], gath[m][:, 0:E], Tanh)
        A_th[m] = th

    def emit_A_tr(m):
        th = A_th.pop(m)
        pst = pAps2.tile([128, 384], f32, tag="pst", name="pst")
        tr(pst[0:128, 0:128], th[:, 0:128], 128)
        tr(pst[0:128, 128:256], th[:, 128:256], 128)
        tr(pst[0:44, 256:384], th[:, 256:300], 128)
        embT_a = pA.tile([128, 256], bf16, tag="embTa", name="embTa")
        nc.vector.tensor_copy(embT_a[:], pst[:, 0:256])
        embT_b = pA.tile([48, 128], bf16, tag="embTb", name="embTb")
        nc.vector.tensor_copy(embT_b[0:44, :], pst[0:44, 256:384])
        emb_sb[m] = (embT_a, embT_b)

    def emit_A_nb_mm(m, nb):
        embT_a, embT_b = emb_sb[m]
        nsl = slice(512 * nb, 512 * (nb + 1))
        psx = pApsx.tile([128, 512], f32, tag="psx", name="psx")
        nc.tensor.matmul(psx[:], embT_a[:, 0:128], wih_sb[0][:, nsl],
                         start=True, stop=False)
        nc.tensor.matmul(psx[:], embT_a[:, 128:256], wih_sb[1][:, nsl],
                         start=False, stop=False)
        nc.tensor.matmul(psx[:], embT_b[0:44, :], wih_sb[2][0:44, nsl],
                         start=False, stop=False)
        nc.tensor.matmul(psx[:], onesr[:, 0:128], biasf[0:1, nsl],
                         start=False, stop=True)
        return psx

    def emit_A_nb_copy(m, nb, psx):
        nsl = slice(512 * nb, 512 * (nb + 1))
        if nb % 2 == 0:
            nc.vector.tensor_copy(xp_tiles[m][:, nsl], psx[:])
        else:
            nc.scalar.activation(xp_tiles[m][:, nsl], psx[:], Ident)

    def emit_junk(n, jpool=None):
        # dense warm-keeper burst: one accumulation group so the MMs stream
        # back-to-back (start=True per-MM would serialize on bank drain)
        jp = (jpool or pJ).tile([128, 512], f32, tag="junk", name="junk")
        for k in range(n):
            nc.tensor.matmul(jp[:], identb[:], junkW[:],
                             start=(k == 0), stop=(k == n - 1))

    # ======== lead-in: A(0), A(1) full + backward cell ========
    xp_tiles = {}
    for m in (0, 1, 2):
        gath[m] = emit_gather(m)
    for m in (0, 1):
        xp_tiles[m] = pXP.tile([128, G4], bf16, tag="xp", name="xp")
        emit_A_tanh(m)
        emit_A_tr(m)
        for nb in range(4):
            psx = emit_A_nb_mm(m, nb)
            emit_A_nb_copy(m, nb, psx)

    # backward LSTM single cell on x_0 (h=c=0), reusing pPS banks
    emb0_a, emb0_b = emb_sb[0]
    bps = pPS.tile([128, 512], f32, tag="psg", name="bps")
    for j in range(4):
        ns = slice(512 * j, 512 * (j + 1))
        o = bps[32 * j:32 * j + 16, :]
        tp = (0, 32 * j)
        nc.tensor.matmul(o, emb0_a[:, 0:16], bwih_sb[0][:, ns],
                         start=True, stop=False, tile_position=tp)
        nc.tensor.matmul(o, emb0_a[:, 128:144], bwih_sb[1][:, ns],
                         start=False, stop=False, tile_position=tp)
        nc.tensor.matmul(o, emb0_b[0:44, 0:16], bwih_sb[2][0:44, ns],
                         start=False, stop=False, tile_position=tp)
        nc.tensor.matmul(o, onesr[:, 0:16], biasb[0:1, ns],
                         start=False, stop=True, tile_position=tp)
    bT = pA.tile([128, 512], f32, tag="bT", name="bT")
    nc.scalar.activation(bT[0:112, :], bps[0:112, :], Tanh)
    bpt = pPS.tile([128, 448], f32, tag="pstT", name="bpt")
    for k in range(4):
        tr(bpt[:, 112 * k:112 * (k + 1)], bT[0:112, 128 * k:128 * (k + 1)], 112)
    bv = bpt[:].rearrange("p (c w) -> p c w", w=112)
    btip = pA.tile([128, 64], f32, tag="btip", name="btip")
    nc.vector.tensor_scalar_add(
        out=btip[:].rearrange("p (c w) -> p c w", w=16),
        in0=bv[:, :, 0:16], scalar1=1.0)
    bzv = pA.tile([128, 64], f32, tag="bzv", name="bzv")
    nc.vector.tensor_tensor(
        out=bzv[:].rearrange("p (c w) -> p c w", w=16),
        in0=btip[:].rearrange("p (c w) -> p c w", w=16),
        in1=bv[:, :, 64:80], op=MUL)
    btc = pA.tile([128, 64], f32, tag="btc", name="btc")
    nc.scalar.activation(btc[:], bzv[:], Tanh, scale=0.5)
    nc.vector.scalar_tensor_tensor(
        out=bH[:].rearrange("p (c w) -> p c w", w=16),
        in0=bv[:, :, 96:112], scalar=1.0,
        in1=btc[:].rearrange("p (c w) -> p c w", w=16),
        op0=ADD, op1=MUL)

    # ======== forward scan, 256 steps, phase A interleaved ========
    z_prev = pZ.tile([128, 64], f32, tag="z", name="z")
    H_prev = pH.tile([128, 64], bf16, tag="H", name="H")
    nc.vector.memset(z_prev[:], 0.0)
    nc.vector.memset(H_prev[:], 0.0)
    emit_junk(10)

    def emit_stage(t):
        st = pST.tile([16, G4], bf16, tag="st", name="st")
        sub = t % 8
        nc.sync.dma_start(st[:], xp_tiles[t // 8][16 * sub:16 * sub + 16, :])
        return st

    stage = {t: emit_stage(t) for t in range(4)}

    for t in range(S):
        m = t // 8
        sub = t % 8
        if m + 2 <= 31:
            if sub == 0 and m + 3 <= 31:
                gath[m + 3] = emit_gather(m + 3)
            elif sub == 1:
                xp_tiles[m + 2] = pXP.tile([128, G4], bf16, tag="xp", name="xp")
                emit_A_tanh(m + 2)
                emit_A_tr(m + 2)
            elif 2 <= sub <= 5:
                psx = emit_A_nb_mm(m + 2, sub - 2)
                emit_A_nb_copy(m + 2, sub - 2, psx)

        xp_src = stage.pop(t)
        if t + 4 < S:
            stage[t + 4] = emit_stage(t + 4)
        psg = pPS.tile([128, 512], f32, tag="psg", name="psg")
        for j in range(4):
            nc.tensor.matmul(
                psg[32 * j:32 * j + 16, :], i16[:],
                xp_src[:, 512 * j:512 * (j + 1)],
                start=True, stop=False, tile_position=(0, 32 * j))
        for kc in range(4):
            for j in range(4):
                nc.tensor.matmul(
                    psg[32 * j:32 * j + 16, :],
                    H_prev[:, 16 * kc:16 * (kc + 1)],
                    whh_sb[kc][:, 512 * j:512 * (j + 1)],
                    start=False, stop=(kc == 3), tile_position=(0, 32 * j))
        emit_junk(DN)

        T_t = pT.tile([128, 512], f32, tag="T", name="T")
        nc.scalar.activation(T_t[0:112, :], psg[0:112, :], Tanh)
        pstT = pPS.tile([128, 448], f32, tag="pstT", name="pstT")
        for k in range(4):
            tr(pstT[:, 112 * k:112 * (k + 1)], T_t[0:112, 128 * k:128 * (k + 1)], 112)
        emit_junk(DN2)

        Tv = pstT[:].rearrange("p (c w) -> p c w", w=112)
        ti, tf = Tv[:, :, 0:16], Tv[:, :, 32:48]
        tg, to = Tv[:, :, 64:80], Tv[:, :, 96:112]

        a_t = pW.tile([128, 64], f32, tag="a", name="a")
        v_t = pW.tile([128, 64], f32, tag="v", name="v")
        av = a_t[:].rearrange("p (c w) -> p c w", w=16)
        vv = v_t[:].rearrange("p (c w) -> p c w", w=16)
        zpv = z_prev[:].rearrange("p (c w) -> p c w", w=16)
        nc.vector.scalar_tensor_tensor(out=av, in0=tf, scalar=1.0, in1=zpv,
                                       op0=ADD, op1=MUL)
        tip = pW.tile([128, 64], f32, tag="tip", name="tip")
        tipv = tip[:].rearrange("p (c w) -> p c w", w=16)
        nc.vector.tensor_scalar_add(out=tipv, in0=ti, scalar1=1.0)
        nc.vector.tensor_tensor(out=vv, in0=tipv, in1=tg, op=MUL)
        z_new = pZ.tile([128, 64], f32, tag="z", name="z")
        nc.vector.scalar_tensor_tensor(out=z_new[:], in0=a_t[:], scalar=0.5,
                                       in1=v_t[:], op0=MUL, op1=ADD)
        tc_t = pW.tile([128, 64], f32, tag="tc", name="tc")
        nc.scalar.activation(tc_t[:], z_new[:], Tanh, scale=0.5)
        H_new = pH.tile([128, 64], bf16, tag="H", name="H")
        nc.vector.scalar_tensor_tensor(
            out=H_new[:].rearrange("p (c w) -> p c w", w=16),
            in0=to, scalar=1.0,
            in1=tc_t[:].rearrange("p (c w) -> p c w", w=16),
            op0=ADD, op1=MUL)
        z_prev, H_prev = z_new, H_new

    pJ.release()
    pPS.release()
    pApsx.release()
    pAps2.release()
    pW.release()
    pZ.release()
    pT.release()
    pST.release()
    pXP.release()
    pA.release()

    # ======== decoder (6 GRU steps + logits + log_softmax) ========
    pD = pool(name="pD", bufs=1)
    pDgi = pool(name="pDgi", bufs=1, space="PSUM")

    ce_t = pD.tile([NCLS, H], f32, tag="ce", name="ce")
    nc.sync.dma_start(ce_t[:], ecw_d[:])
    ce2 = pD.tile([NCLS, H], f32, tag="ce2", name="ce2")
    nc.scalar.activation(ce2[:], ce_t[:], Tanh)
    psc = pDgi.tile([128, 24], f32, tag="psc", name="psc")
    for k in range(4):
        tr(psc[:, 6 * k:6 * (k + 1)], ce2[0:NCLS, 128 * k:128 * (k + 1)], NCLS)
    ceT = pD.tile([128, 24], bf16, tag="ceT", name="ceT")
    nc.vector.tensor_copy(ceT[:], psc[:])

    psgi = pDgi.tile([NCLS, G3], f32, tag="psgi", name="psgi")
    for ng in range(6):
        ns = slice(512 * ng, 512 * (ng + 1))
        for kc in range(4):
            nc.tensor.matmul(psgi[:, ns], ceT[:, 6 * kc:6 * (kc + 1)],
                             dwih_sb[kc][:, ns], start=(kc == 0), stop=False)
        nc.tensor.matmul(psgi[:, ns], bias_stat[0:4, 0:NCLS],
                         dwih_sb[4][0:4, ns], start=False, stop=True)
    gi_sb = pD.tile([NCLS, G3], bf16, tag="gi", name="gi")
    nc.scalar.activation(gi_sb[:], psgi[:], Ident)

    # transposed gi_n (per-partition bias for the n-gate)
    psgT = pDgi.tile([128, 48], bf16, tag="psgT", name="psgT")
    for gc in range(8):
        nc.tensor.transpose(psgT[:, 6 * gc:6 * (gc + 1)],
                            gi_sb[0:NCLS, 2048 + 128 * gc:2048 + 128 * (gc + 1)],
                            identb[0:NCLS, 0:NCLS])
    giT = pD.tile([128, 48], f32, tag="giT", name="giT")
    nc.vector.tensor_copy(giT[:], psgT[:])

    # per-class gi rows at partition 0, via DRAM bounce (avoids recompute)
    gi_dram = dramp.tile([NCLS, G3], bf16, tag="gid", name="gid")
    nc.sync.dma_start(gi_dram[:], gi_sb[:])
    gi_row = pD.tile([1, NCLS * G3], bf16, tag="girow", name="girow")
    for c in range(NCLS):
        nc.sync.dma_start(gi_row[0:1, G3 * c:G3 * (c + 1)],
                          gi_dram[c:c + 1, :])
    pDgi.release()
    pDps = pool(name="pDps", bufs=1, space="PSUM")
    pJ2 = pool(name="pJ2", bufs=1, space="PSUM")

    Hd = pD.tile([128, 128], bf16, tag="Hd", name="Hd")
    nc.vector.tensor_scalar_mul(Hd[:, 0:64], H_prev[:], 0.5)
    nc.vector.tensor_scalar_mul(Hd[:, 64:128], bH[:], 0.5)

    l_all = pD.tile([16, 12], f32, tag="lall", name="lall")

    for c in range(NCLS):
        psd0 = pDps.tile([128, 512], f32, tag="psd0", name="psd0")
        psd1 = pDps.tile([128, 512], f32, tag="psd1", name="psd1")
        for kc in range(8):
            lh = Hd[:, 16 * kc:16 * (kc + 1)]
            for ng in range(6):
                ps, j = (psd0, ng) if ng < 4 else (psd1, ng - 4)
                nc.tensor.matmul(
                    ps[32 * j:32 * j + 16, :], lh,
                    dwhh_sb[kc][:, 512 * ng:512 * (ng + 1)],
                    start=(kc == 0), stop=False, tile_position=(0, 32 * j))
        for ng in range(6):
            ps, j = (psd0, ng) if ng < 4 else (psd1, ng - 4)
            nc.tensor.matmul(
                ps[32 * j:32 * j + 16, :], bias_stat[0:4, :],
                dwhh_sb[8][0:4, 512 * ng:512 * (ng + 1)],
                start=False, stop=(ng >= 4), tile_position=(0, 32 * j))
        for ng in range(4):
            nc.tensor.matmul(
                psd0[32 * ng:32 * ng + 16, :], ones1[:],
                gi_row[0:1, G3 * c + 512 * ng:G3 * c + 512 * (ng + 1)],
                start=False, stop=True, tile_position=(0, 32 * ng))

        Trz = pD.tile([128, 512], f32, tag="Trz", name="Trz")
        nc.scalar.activation(Trz[0:112, :], psd0[0:112, :], Tanh)
        pstz = pDps.tile([128, 448], f32, tag="pstz", name="pstz")
        for k in range(4):
            tr(pstz[:, 112 * k:112 * (k + 1)], Trz[0:112, 128 * k:128 * (k + 1)], 112)
        hn_sb = pD.tile([48, 512], f32, tag="hn", name="hn")
        nc.vector.tensor_copy(hn_sb[:], psd1[0:48, :])
        psn = pDps.tile([128, 192], f32, tag="psn", name="psn")
        for k in range(4):
            tr(psn[:, 48 * k:48 * (k + 1)], hn_sb[0:48, 128 * k:128 * (k + 1)], 48)
        emit_junk(10, pJ2)

        zv = pstz[:].rearrange("p (c w) -> p c w", w=112)
        nv = psn[:].rearrange("p (c w) -> p c w", w=48)
        trp = pD.tile([128, 128], f32, tag="trp", name="trp")
        trpv = trp[:].rearrange("p (g w) -> p g w", w=16)
        sn_t = pD.tile([128, 128], f32, tag="sn", name="sn")
        snv = sn_t[:].rearrange("p (g w) -> p g w", w=16)
        nT = pD.tile([128, 128], f32, tag="nT", name="nT")
        for s in range(2):
            nc.vector.tensor_scalar_add(
                out=trpv[:, 4 * s:4 * s + 4, :],
                in0=zv[:, :, 32 * s:32 * s + 16], scalar1=1.0)
            nc.vector.tensor_tensor(
                out=snv[:, 4 * s:4 * s + 4, :],
                in0=trpv[:, 4 * s:4 * s + 4, :],
                in1=nv[:, :, 32 * s:32 * s + 16], op=MUL)
        nTv = nT[:].rearrange("p (g w) -> p g w", w=16)
        for g in range(8):
            nc.scalar.activation(nTv[:, g, :], snv[:, g, :], Tanh, scale=0.5,
                                 bias=giT[:, 6 * g + c:6 * g + c + 1])
        d_t = pD.tile([128, 128], f32, tag="dt", name="dt")
        nc.vector.tensor_tensor(out=d_t[:], in0=Hd[:], in1=nT[:], op=SUB)
        e_t = pD.tile([128, 128], f32, tag="et", name="et")
        ev = e_t[:].rearrange("p (g w) -> p g w", w=16)
        dv = d_t[:].rearrange("p (g w) -> p g w", w=16)
        for s in range(2):
            nc.vector.scalar_tensor_tensor(
                out=ev[:, 4 * s:4 * s + 4, :],
                in0=zv[:, :, 64 + 32 * s:80 + 32 * s], scalar=1.0,
                in1=dv[:, 4 * s:4 * s + 4, :], op0=ADD, op1=MUL)
        hn2 = pD.tile([128, 128], f32, tag="hn2", name="hn2")
        nc.vector.scalar_tensor_tensor(out=hn2[:], in0=e_t[:], scalar=0.5,
                                       in1=nT[:], op0=MUL, op1=ADD)
        Hd_new = pD.tile([128, 128], bf16, tag="Hd", name="Hd")
        nc.scalar.activation(Hd_new[:], hn2[:], Tanh)

        psl = pDps.tile([16, 2], f32, tag="psl", name="psl")
        for kc in range(8):
            nc.tensor.matmul(psl[:], Hd_new[:, 16 * kc:16 * (kc + 1)],
                             cls_sb[kc][:, 0:2], start=(kc == 0), stop=False)
        nc.tensor.matmul(psl[:], bias_stat[0:4, :], cls_sb[8][0:4, 0:2],
                         start=False, stop=True)
        nc.vector.tensor_copy(l_all[:, 2 * c:2 * c + 2], psl[:])
        Hd = Hd_new

    la = l_all[:].rearrange("p (c t) -> p c t", t=2)
    mx = pD.tile([16, 6], f32, tag="mx", name="mx")
    nc.vector.tensor_tensor(out=mx[:].rearrange("p (c o) -> p c o", o=1),
                            in0=la[:, :, 0:1], in1=la[:, :, 1:2], op=MAX)
    d0 = pD.tile([16, 12], f32, tag="d0", name="d0")
    d0v = d0[:].rearrange("p (c t) -> p c t", t=2)
    mxb = mx[:].rearrange("p (c o) -> p c o", o=1).to_broadcast([16, 6, 2])
    nc.vector.tensor_tensor(out=d0v, in0=la, in1=mxb, op=SUB)
    ex = pD.tile([16, 12], f32, tag="ex", name="ex")
    nc.scalar.activation(ex[:], d0[:], Exp)
    se = pD.tile([16, 6], f32, tag="se", name="se")
    nc.vector.tensor_reduce(out=se[:].rearrange("p (c o) -> p c o", o=1),
                            in_=ex[:].rearrange("p (c t) -> p c t", t=2),
                            op=ADD, axis=mybir.AxisListType.X)
    ls = pD.tile([16, 6], f32, tag="ls", name="ls")
    nc.scalar.activation(ls[:], se[:], Ln)
    ov = pD.tile([16, 12], f32, tag="ov", name="ov")
    lsb = ls[:].rearrange("p (c o) -> p c o", o=1).to_broadcast([16, 6, 2])
    nc.vector.tensor_tensor(out=ov[:].rearrange("p (c t) -> p c t", t=2),
                            in0=d0v, in1=lsb, op=SUB)
    nc.sync.dma_start(out_d[:].rearrange("c b t -> b c t"),
                      ov[:].rearrange("p (c t) -> p c t", t=2))

    pJ2.release()
    pDps.release()
    pD.release()
    pH.release()
    dramp.release()
    const.release()


def _prep_inputs(seq, classes, embed_W, embed_class_W, f_Wih, f_Whh, f_b,
                 b_Wih, b_Whh, b_b, d_Wih, d_Whh, d_bih, d_bhh, cls_W, cls_b):
    seq = np.asarray(seq)
    s4 = np.concatenate([np.full(H, 0.5), np.full(H, 0.5), np.ones(H),
                         np.full(H, 0.5)]).astype(np.float32)
    s3 = np.concatenate([np.full(H2, 0.5), np.full(H2, 0.5),
                         np.ones(H2)]).astype(np.float32)

    def padrows(a, rows):
        out = np.zeros((rows, a.shape[1]), np.float32)
        out[:a.shape[0]] = a
        return out

    wihT = padrows(np.concatenate(
        [(f_Wih * s4[:, None]).T, (f_b * s4)[None, :]], axis=0), 304)
    bwihT = padrows(np.concatenate(
        [(b_Wih * s4[:, None]).T, (b_b * s4)[None, :]], axis=0), 304)
    whhT = ((f_Whh * s4[:, None]) * 0.5).T.astype(np.float32)
    dwhhT = padrows(np.concatenate(
        [(d_Whh * s3[:, None]).T, (d_bhh * s3)[None, :]], axis=0), 1028)
    dwihT = padrows(np.concatenate(
        [(d_Wih * s3[:, None]).T, (d_bih * s3)[None, :]], axis=0), 516)
    clsT = padrows(np.concatenate(
        [np.asarray(cls_W, np.float32).T, np.asarray(cls_b, np.float32)[None, :]],
        axis=0), 1028)
    ecw = np.asarray(embed_class_W, np.float32)[np.asarray(classes)]

    shared = {
        "embedW": np.ascontiguousarray(np.asarray(embed_W, np.float32)),
        "wihT": _bf(wihT), "bwihT": _bf(bwihT), "whhT": _bf(whhT),
        "dwhhT": _bf(dwhhT), "dwihT": _bf(dwihT),
        "ecw": np.ascontiguousarray(ecw),
        "clsT": _bf(clsT),
    }
    in_maps = []
    for c in range(NC):
        tok = np.asarray(seq[BL * c:BL * (c + 1), :], np.int32)  # [16, 256]
        idx = np.ascontiguousarray(
            tok.T.reshape(S * BL).reshape(32, 128).T.astype(np.int32))
        m = dict(shared)
        m["idx"] = idx
        in_maps.append(m)
    return in_maps


def kernel(**inputs):
    if "nc" not in _cache:
        _cache["nc"] = _build_program()
    nc = _cache["nc"]
    in_maps = _prep_inputs(**inputs)
    import os
    trace = bool(int(os.environ.get("BK_TRACE", "0")))
    res = run_bass_kernel_spmd(nc, in_maps, core_ids=list(range(NC)),
                               trace=trace)
    _cache["last_result"] = res
    outs = [res.results[c]["out"] for c in range(NC)]
    return np.concatenate(outs, axis=1).astype(np.float32)
